# revision 1
# baseline (speedup 1.0000x reference)
"""Bag-of-words histogram kernel for Trainium2 (Bass/Tile), 8-core data-parallel.

Problem: docs [256, 2048] int32 token ids in [0, 32000) ->
         hist [256, 32000] fp32, hist[b, v] = count(docs[b, :] == v) / 2048.

Algorithm (per core, 32 rows):
  Factor each token t = 256*hi + lo (hi < 125, lo < 256). Then
    hist[b, hi, lo] = sum_s onehot_hi[s, hi] * onehot_lo[s, lo]
  i.e. a [128-token] x [128 hi] x [256 lo] outer-product matmul accumulated
  over 16 K-tiles per row on the PE, with the one-hots built by DVE
  tensor_scalar is_equal against iota constants (bf16, 4x DVE mode).
  PSUM accumulates exact integer counts in fp32; the PSUM->SBUF copyback
  applies the exact 2^-11 scaling on the scalar engine.

Sharding: batch axis split 8 ways (32 rows per core), no communication.
"""

import sys

import numpy as np

for _p in ("/opt/trn_rl_repo",):
    if _p not in sys.path:
        sys.path.append(_p)

BATCH = 256
SEQ = 2048
VOCAB = 32000
N_CORES = 8
ROWS = BATCH // N_CORES  # 32 rows per core
P = 128
KT = SEQ // P            # 16 K-tiles per row
GR = 8                   # rows per input-DMA group
NLO = 256                # low-digit bins (t & 255)
NHI = 128                # high-digit compare width (t >> 8 < 125)

# Engine per (row parity, k-tile) for the hi-one-hot build: "V" = vector
# (DVE) tensor_scalar, "A" = scalar (ACT) Square+Relu pair, "G" = gpsimd
# tensor_scalar (measured ~2.3us/op - do not use). Tuned from
# neuron-profile runs: DVE effective ~182ns/op, ACT ~388ns/op serial and
# the hi build needs two ACT ops, so ACT carries ~1/3 of the hi builds.
_ACT_K = {0: (2, 5, 8, 11, 14), 1: (2, 5, 8, 11, 14)}
HI_ENGINE = {(par, k): ("A" if k in _ACT_K[par] else "V")
             for par in (0, 1) for k in range(KT)}
ACT_HI_KTILES = frozenset(k for (_, k), e in HI_ENGINE.items() if e == "A")


def _build_nc():
    from contextlib import ExitStack

    from concourse import bacc, bass, mybir
    from concourse.tile import TileContext

    nc = bacc.Bacc()
    docs = nc.dram_tensor("docs", [ROWS, SEQ], mybir.dt.int32, kind="ExternalInput")
    hist = nc.dram_tensor("hist", [ROWS, VOCAB], mybir.dt.float32, kind="ExternalOutput")

    f32 = mybir.dt.float32
    bf16 = mybir.dt.bfloat16
    Alu = mybir.AluOpType

    with TileContext(nc) as tc, ExitStack() as ctx:
        const_tp = ctx.enter_context(tc.tile_pool(name="const", bufs=1))
        tok_tp = ctx.enter_context(tc.tile_pool(name="tok", bufs=8))
        hilo_tp = ctx.enter_context(tc.tile_pool(name="hilo", bufs=8))
        oh_tp = ctx.enter_context(tc.tile_pool(name="oh", bufs=12))
        res_tp = ctx.enter_context(tc.tile_pool(name="res", bufs=4))
        psum_tp = ctx.enter_context(tc.tile_pool(name="psum", bufs=7, space="PSUM"))

        # iota constants: value = column index, identical on every partition
        iota_hi = const_tp.tile([P, NHI], bf16)
        nc.gpsimd.iota(iota_hi[:], [[1, NHI]], channel_multiplier=0,
                       allow_small_or_imprecise_dtypes=True)
        iota_lo = const_tp.tile([P, NLO], bf16)
        nc.gpsimd.iota(iota_lo[:], [[1, NLO]], channel_multiplier=0,
                       allow_small_or_imprecise_dtypes=True)

        for g in range(ROWS // GR):
            # Load GR rows; partition p holds tokens [16p, 16p+16) of each row
            # (any within-row permutation is histogram-invariant, so a fully
            # contiguous 64B-per-partition-line DMA is used).
            tok = tok_tp.tile([P, GR, KT], mybir.dt.int32)
            src = bass.AP(docs, g * GR * SEQ, [[16, P], [SEQ, GR], [1, KT]])
            nc.sync.dma_start(out=tok[:], in_=src)

            # hi = t >> 8, lo = t & 255; bit-vector ops cannot cast on HW,
            # so shift/and stay int32 and a mult-by-1.0 does the fp32 cast.
            hi_i = hilo_tp.tile([P, GR, KT], mybir.dt.int32, tag="hii")
            lo_i = hilo_tp.tile([P, GR, KT], mybir.dt.int32, tag="loi")
            nc.vector.tensor_scalar(out=hi_i[:], in0=tok[:], scalar1=8,
                                    scalar2=None, op0=Alu.logical_shift_right)
            nc.vector.tensor_scalar(out=lo_i[:], in0=tok[:], scalar1=255,
                                    scalar2=None, op0=Alu.bitwise_and)
            hi_pl = hilo_tp.tile([P, GR, KT], f32, tag="hi")
            lo_pl = hilo_tp.tile([P, GR, KT], f32, tag="lo")
            nc.vector.tensor_scalar(out=hi_pl[:], in0=hi_i[:], scalar1=1.0,
                                    scalar2=None, op0=Alu.mult)
            nc.vector.tensor_scalar(out=lo_pl[:], in0=lo_i[:], scalar1=1.0,
                                    scalar2=None, op0=Alu.mult)
            if ACT_HI_KTILES:
                # negated hi for the ACT Square-bias path
                nhi_pl = hilo_tp.tile([P, GR, KT], f32, tag="nhi")
                nc.vector.tensor_scalar(out=nhi_pl[:], in0=hi_i[:], scalar1=-1.0,
                                        scalar2=None, op0=Alu.mult)

            for rl in range(GR):
                r = g * GR + rl
                ps = psum_tp.tile([P, NLO], f32)
                for k in range(KT):
                    oh_hi = oh_tp.tile([P, NHI], bf16, tag="ohhi")
                    oh_lo = oh_tp.tile([P, NLO], bf16, tag="ohlo")
                    eng = HI_ENGINE[(r % 2, k)]
                    if eng == "A":
                        # (iota - hi)^2 then relu(1 - d2): exact one-hot
                        sq = oh_tp.tile([P, NHI], bf16, tag="sq")
                        nc.scalar.activation(
                            out=sq[:], in_=iota_hi[:],
                            func=mybir.ActivationFunctionType.Square,
                            bias=nhi_pl[:, rl, k:k + 1], scale=1.0)
                        nc.scalar.activation(
                            out=oh_hi[:], in_=sq[:],
                            func=mybir.ActivationFunctionType.Relu,
                            bias=1.0, scale=-1.0)
                    elif eng == "G":
                        nc.gpsimd.tensor_scalar(
                            out=oh_hi[:], in0=iota_hi[:],
                            scalar1=hi_pl[:, rl, k:k + 1], scalar2=None,
                            op0=Alu.is_equal)
                    else:
                        nc.vector.tensor_scalar(
                            out=oh_hi[:], in0=iota_hi[:],
                            scalar1=hi_pl[:, rl, k:k + 1], scalar2=None,
                            op0=Alu.is_equal)
                    nc.vector.tensor_scalar(
                        out=oh_lo[:], in0=iota_lo[:],
                        scalar1=lo_pl[:, rl, k:k + 1], scalar2=None,
                        op0=Alu.is_equal)
                    nc.tensor.matmul(out=ps[:], lhsT=oh_hi[:], rhs=oh_lo[:],
                                     start=(k == 0), stop=(k == KT - 1))

                res = res_tp.tile([P, NLO], f32)
                nc.scalar.mul(out=res[:], in_=ps[:], mul=1.0 / SEQ)
                nc.sync.dma_start(
                    out=hist[r].rearrange("(h l) -> h l", l=NLO),
                    in_=res[:VOCAB // NLO, :])
    nc.compile()
    return nc


_NC_CACHE = None


def _get_nc():
    global _NC_CACHE
    if _NC_CACHE is None:
        _NC_CACHE = _build_nc()
    return _NC_CACHE


def run_sharded(docs: np.ndarray, trace: bool = False):
    """Run the 8-core SPMD kernel. Returns (full_output, BassKernelResults)."""
    from concourse.bass_utils import run_bass_kernel_spmd

    docs = np.ascontiguousarray(np.asarray(docs, dtype=np.int32))
    assert docs.shape == (BATCH, SEQ), docs.shape
    shards = docs.reshape(N_CORES, ROWS, SEQ)
    in_maps = [{"docs": shards[i]} for i in range(N_CORES)]
    res = run_bass_kernel_spmd(_get_nc(), in_maps, core_ids=list(range(N_CORES)),
                               trace=trace)
    out = np.concatenate([res.results[i]["hist"] for i in range(N_CORES)], axis=0)
    return out, res


def kernel(docs: np.ndarray) -> np.ndarray:
    out, _ = run_sharded(docs, trace=False)
    return out

